# revision 11
# baseline (speedup 1.0000x reference)
"""Blockwise (compressed-KV) attention on 8 Trainium2 NeuronCores.

Problem: q,k,v [B=4,H=16,T=4096,D=128] fp32, BS=32.
  k_cmp/v_cmp = blockwise mean-pool of k/v along T -> [B,H,C=128,D]
  score = softmax(q @ k_cmp^T / sqrt(D))   [B,H,T,C]
  out   = score @ v_cmp                    [B,H,T,D]
Returns (out, score), matching the reference.

Sharding: the 64 (b,h) pairs are split 8-per-core (pure data parallel, no
communication).  Each core runs an identical Bass/Tile program over its
8 heads.

v3 design (per head, per core):
  loads: k,v via SWDGE cast DMA f32->fp16 as [128, 32*128] (partition =
    block index, 16 KiB contiguous per partition in HBM); q f32 as
    [128, 32, 128] on the SP HWDGE ring (one 2 MiB DMA).
  pooling: OFF the PE - 5-round fp16 tree-add on DVE (2x packed mode),
    final round emits f32 sums.  The 1/32 is folded into the exp scale
    (k side) and into the v_cmp fp16 copy (v side).
  k_cmp^T via one PE transpose of ksum; fp16 operands for all matmuls.
  main loop over 8 groups of 512 q rows:
    4 PE transposes q -> qT PSUM (f32), evac to fp16 SBUF (DVE+ACT split),
    QK^T: 4 matmuls (qT tile stationary, k_cmpT moving) -> S [t,c]
    S^T:  1 matmul  (k_cmpT stationary, qT moving, N=512)
    exp via ACT with scale=1/(32 sqrt(D)): S -> fp16 expt, S^T -> fp16 expT
    rowsums: DVE segmented reduce on expt -> sums f32; reciprocal
    normalize: DVE fp16 (4x mode) -> score_stage
    PV: 4 matmuls (expT tile stationary, v_cmp fp16 moving) -> out [t,d]
    evac out with scale=recip (ACT) -> fp16 out_stage
  stores: one 1 MiB fp16 DMA each for score/out per head on the ACT ring,
    DRAM layout [p, s, c] (partition-major, contiguous 8 KiB per partition);
    the host transposes back to [t, c] and upcasts to f32.
"""
import math

import numpy as np

import concourse.bass as bass
import concourse.tile as tile
from concourse import mybir
from concourse.bass_utils import run_bass_kernel_spmd
from concourse.vector_clock import ScopedClock

B, H, T, D = 4, 16, 4096, 128
BS_EXPECTED = 32
C = T // BS_EXPECTED  # 128 compressed slots
N_CORES = 8
HEADS_PER_CORE = B * H // N_CORES  # 8
N_TT = T // 128  # 32 t-tiles of 128 rows per head
F32 = mybir.dt.float32
FP16 = mybir.dt.float16

# ---------------------------------------------------------------------------
# walrus in this toolchain rejects instructions carrying more than one sync
# wait.  Tile's scheduler freely emits several waits per instruction, and the
# kernel-tail drain accumulates one wait per outstanding semaphore.  Hoist all
# but one wait of every instruction onto dedicated same-engine NOPs placed
# immediately before it (same-engine program order keeps the semantics).
_MAX_WAITS = 1
_split_counter = [0]


def _split_multi_waits(ordered):
    for insts in ordered.values():
        expanded = []
        for inst in insts:
            si = inst.sync_info
            if si is not None and len(si.on_wait) > _MAX_WAITS:
                waits = list(si.on_wait)
                head, keep = waits[:-_MAX_WAITS], waits[-_MAX_WAITS:]
                for w in head:
                    _split_counter[0] += 1
                    expanded.append(mybir.InstNoOp(
                        name=f"waitsplit_{_split_counter[0]}",
                        ins=[], outs=[],
                        engine=inst.engine,
                        sync_info=mybir.SyncInfo(on_wait=[w], on_update=[]),
                        bass_nofuse=True,
                    ))
                inst.sync_info = mybir.SyncInfo(
                    on_wait=keep, on_update=list(si.on_update)
                )
            expanded.append(inst)
        insts[:] = expanded


_orig_lower_ordered = tile.TileContext._lower_ordered_insts


def _lower_ordered_split(self, ordered):
    _split_multi_waits(ordered)
    return _orig_lower_ordered(self, ordered)


tile.TileContext._lower_ordered_insts = _lower_ordered_split


def _drain_and_barrier_split(self, tick_clock, wait_clock):
    nc = self.nc
    drain_inst = nc.sync.drain()
    wait_clock.add_sem_waits(
        drain_inst.ins, ScopedClock({None: tick_clock.global_clock})
    )
    si = drain_inst.ins.sync_info
    waits = list(si.on_wait) if si is not None else []
    if len(waits) > _MAX_WAITS:
        drain_inst.ins.sync_info = mybir.SyncInfo(
            on_wait=waits[:_MAX_WAITS], on_update=list(si.on_update)
        )
        for i in range(_MAX_WAITS, len(waits), _MAX_WAITS):
            extra = nc.sync.drain()
            extra.ins.sync_info = mybir.SyncInfo(
                on_wait=waits[i : i + _MAX_WAITS], on_update=[]
            )
    nc.all_engine_barrier()
    assert self.sems is not None
    popped = nc._tile_sem_poison_stack.pop()
    assert popped is self._sem_poison
    nc.clear_and_free_semaphores(list(self.sems.allocated().values()))
    nc.all_engine_barrier()


tile.TileContext._drain_and_barrier = _drain_and_barrier_split
# ---------------------------------------------------------------------------


def _tree_pool(nc, pool, src16, final_dtype, tag):
    """5-round fp16 pairwise-add tree: [128, 32*D] -> [128, D] sum."""
    cur = src16
    n = 16 * D
    while n > D:
        nxt = pool.tile([128, n], FP16, tag=f"{tag}{n}")
        nc.vector.tensor_tensor(
            nxt, cur[:, 0:n], cur[:, n : 2 * n], mybir.AluOpType.add
        )
        cur = nxt
        n //= 2
    out = pool.tile([128, D], final_dtype, tag=f"{tag}f")
    nc.vector.tensor_tensor(
        out, cur[:, 0:D], cur[:, D : 2 * D], mybir.AluOpType.add
    )
    return out


def build_program(reps: int = 1, cast_loads: bool = True, q16: bool = True,
                  sums_on: str = "pe", norm_on: str = "split",
                  dma_only: bool = False) -> bass.Bass:
    """Build the per-core Bass program.  `reps` repeats the whole computation
    (identical work, same outputs) for slope-based wall-clock timing."""
    nc = bass.Bass("TRN2", target_bir_lowering=False, debug=False,
                   num_devices=N_CORES)

    q_d = nc.dram_tensor("q", [HEADS_PER_CORE, T, D], F32, kind="ExternalInput").ap()
    k_d = nc.dram_tensor("k", [HEADS_PER_CORE, T, D], F32, kind="ExternalInput").ap()
    v_d = nc.dram_tensor("v", [HEADS_PER_CORE, T, D], F32, kind="ExternalInput").ap()
    ident_d = nc.dram_tensor("ident", [128, 128], F32, kind="ExternalInput").ap()
    # [p, s, c] partition-major layout; host transposes back to [t, c].
    out_d = nc.dram_tensor("out", [HEADS_PER_CORE, 128, N_TT, D], FP16,
                           kind="ExternalOutput").ap()
    score_d = nc.dram_tensor("score", [HEADS_PER_CORE, 128, N_TT, C], FP16,
                             kind="ExternalOutput").ap()

    inv_scale = 1.0 / (BS_EXPECTED * math.sqrt(D))  # 1/32 pool fold + 1/sqrt(d)

    with tile.TileContext(nc) as tc:
        with (
            tc.tile_pool(name="singles", bufs=1) as singles,
            tc.tile_pool(name="kv", bufs=2) as kv_pool,
            tc.tile_pool(name="tree", bufs=2) as tree_pool,
            tc.tile_pool(name="qp", bufs=2) as q_pool,
            tc.tile_pool(name="heads", bufs=2) as heads,
            tc.tile_pool(name="sb", bufs=4) as sb_pool,
            tc.tile_pool(name="stage", bufs=2) as stage,
            tc.tile_pool(name="small", bufs=4) as small_pool,
            tc.tile_pool(name="psA", bufs=2, space="PSUM") as psA,
            tc.tile_pool(name="psS", bufs=2, space="PSUM") as psS,
            tc.tile_pool(name="psST", bufs=2, space="PSUM") as psST,
            tc.tile_pool(name="psO", bufs=2, space="PSUM") as psO,
        ):
            ident = singles.tile([128, 128], F32)
            nc.sync.dma_start(out=ident, in_=ident_d)
            ident16 = singles.tile([128, 128], FP16)
            nc.vector.tensor_copy(ident16, ident)
            ones16 = singles.tile([128, 1], FP16)
            nc.vector.memset(ones16, 1.0)
            q_dt = FP16 if q16 else F32
            q_ident = ident16 if q16 else ident

            for _rep in range(reps):
                for h in range(HEADS_PER_CORE):
                    # ---- loads ------------------------------------------
                    if cast_loads:
                        k_sb = kv_pool.tile([128, BS_EXPECTED * D], FP16, tag="kv")
                        nc.gpsimd.dma_start(
                            out=k_sb,
                            in_=k_d[h].rearrange("(p j) d -> p (j d)", p=128),
                        )
                        v_sb = kv_pool.tile([128, BS_EXPECTED * D], FP16, tag="kv")
                        nc.gpsimd.dma_start(
                            out=v_sb,
                            in_=v_d[h].rearrange("(p j) d -> p (j d)", p=128),
                        )
                    else:
                        k_sb = kv_pool.tile([128, BS_EXPECTED * D], F32, tag="kv32")
                        nc.sync.dma_start(
                            out=k_sb,
                            in_=k_d[h].rearrange("(p j) d -> p (j d)", p=128),
                        )
                        v_sb = kv_pool.tile([128, BS_EXPECTED * D], F32, tag="kv32")
                        nc.sync.dma_start(
                            out=v_sb,
                            in_=v_d[h].rearrange("(p j) d -> p (j d)", p=128),
                        )
                    # contiguous load: partition p holds rows t = 32p..32p+31,
                    # so tile j is q rows {32p + j} and every downstream
                    # [p, j] layout is linear t-order (t = 32p + j).
                    q_sb = q_pool.tile([128, N_TT, D], q_dt, tag="q")
                    if q16:
                        nc.gpsimd.dma_start(
                            out=q_sb,
                            in_=q_d[h].rearrange("(p j) d -> p j d", p=128),
                        )
                    else:
                        nc.sync.dma_start(
                            out=q_sb,
                            in_=q_d[h].rearrange("(p j) d -> p j d", p=128),
                        )

                    # ---- pooling (DVE) ----------------------------------
                    if cast_loads:
                        ksum = _tree_pool(nc, tree_pool, k_sb, F32, "k")
                        vsum = _tree_pool(nc, tree_pool, v_sb, F32, "v")
                    else:
                        ksum = heads.tile([128, D], F32, tag="ks")
                        nc.vector.reduce_sum(
                            ksum,
                            k_sb.rearrange("p (j d) -> p d j",
                                           j=BS_EXPECTED, d=D),
                            axis=mybir.AxisListType.X,
                        )
                        vsum = heads.tile([128, D], F32, tag="vs")
                        nc.vector.reduce_sum(
                            vsum,
                            v_sb.rearrange("p (j d) -> p d j",
                                           j=BS_EXPECTED, d=D),
                            axis=mybir.AxisListType.X,
                        )
                    # v_cmp = vsum/32 in fp16 (folds the mean)
                    v_cmp = heads.tile([128, D], FP16, tag="vc")
                    nc.scalar.activation(
                        v_cmp, vsum, mybir.ActivationFunctionType.Copy,
                        scale=1.0 / BS_EXPECTED,
                    )
                    kt_ps = psA.tile([128, 512], F32, tag="a")
                    nc.tensor.transpose(kt_ps[:, 0:128], ksum, ident)
                    k_cmpT = heads.tile([128, C], FP16, tag="kc")
                    nc.scalar.copy(k_cmpT, kt_ps[:, 0:128])

                    score_stage = stage.tile([128, N_TT, C], FP16, tag="sc")
                    out_stage = stage.tile([128, N_TT, D], FP16, tag="ou")

                    if dma_only:
                        nc.vector.memset(score_stage[:, 0:1, 0:1], 0.5)
                        nc.vector.memset(out_stage[:, 0:1, 0:1], 0.25)
                    else:
                        for g in range(N_TT // 4):  # 8 groups of 512 rows
                            qT_ps = psA.tile([128, 512], q_dt, tag="a")
                            for j in range(4):
                                nc.tensor.transpose(
                                    qT_ps[:, 128 * j : 128 * (j + 1)],
                                    q_sb[:, 4 * g + j, :], q_ident,
                                )
                            qT = sb_pool.tile([128, 512], FP16, tag="qT")
                            nc.vector.tensor_copy(qT[:, 0:256], qT_ps[:, 0:256])
                            nc.scalar.copy(qT[:, 256:512], qT_ps[:, 256:512])

                            s_ps = psS.tile([128, 512], F32, tag="s")
                            for j in range(4):
                                nc.tensor.matmul(
                                    s_ps[:, 128 * j : 128 * (j + 1)],
                                    lhsT=qT[:, 128 * j : 128 * (j + 1)],
                                    rhs=k_cmpT,
                                    start=True, stop=True,
                                )
                            stp_ps = psST.tile([128, 512], F32, tag="st")
                            nc.tensor.matmul(
                                stp_ps, lhsT=k_cmpT, rhs=qT,
                                start=True, stop=True,
                            )
                            expt = sb_pool.tile([128, 512], FP16, tag="exp")
                            nc.scalar.activation(
                                expt, s_ps, mybir.ActivationFunctionType.Exp,
                                scale=inv_scale,
                            )
                            expT = sb_pool.tile([128, 512], FP16, tag="st2")
                            nc.scalar.activation(
                                expT, stp_ps, mybir.ActivationFunctionType.Exp,
                                scale=inv_scale,
                            )
                            recip = small_pool.tile([128, 4], F32, tag="recip")
                            if sums_on == "pe":
                                sums_ps = psST.tile([128, 4], F32, tag="st")
                                for j in range(4):
                                    nc.tensor.matmul(
                                        sums_ps[:, j : j + 1],
                                        lhsT=expT[:, 128 * j : 128 * (j + 1)],
                                        rhs=ones16,
                                        start=True, stop=True,
                                    )
                                nc.vector.reciprocal(recip, sums_ps)
                            else:
                                sums = small_pool.tile([128, 4], F32, tag="sums")
                                nc.vector.reduce_sum(
                                    sums,
                                    expt.rearrange("p (j c) -> p j c", j=4),
                                    axis=mybir.AxisListType.X,
                                )
                                nc.vector.reciprocal(recip, sums)

                            for j in range(4):
                                norm_eng = nc.vector
                                if norm_on == "gps" or (
                                    norm_on == "split" and j >= 2
                                ):
                                    norm_eng = nc.gpsimd
                                norm_eng.tensor_scalar_mul(
                                    score_stage[:, 4 * g + j, :],
                                    expt[:, 128 * j : 128 * (j + 1)],
                                    recip[:, j : j + 1],
                                )

                            o_ps = psO.tile([128, 512], F32, tag="o")
                            for j in range(4):
                                nc.tensor.matmul(
                                    o_ps[:, 128 * j : 128 * (j + 1)],
                                    lhsT=expT[:, 128 * j : 128 * (j + 1)],
                                    rhs=v_cmp,
                                    start=True, stop=True,
                                )
                            for j in range(4):
                                nc.scalar.activation(
                                    out_stage[:, 4 * g + j, :],
                                    o_ps[:, 128 * j : 128 * (j + 1)],
                                    mybir.ActivationFunctionType.Copy,
                                    scale=recip[:, j : j + 1],
                                )

                    # ---- stores (ACT ring), contiguous per partition ----
                    nc.scalar.dma_start(out=score_d[h], in_=score_stage)
                    nc.scalar.dma_start(out=out_d[h], in_=out_stage)
    return nc


def _make_const_inputs():
    ident = np.eye(128, dtype=np.float32)
    pmat = np.zeros((128, 4), dtype=np.float32)
    for t in range(128):
        pmat[t, t // 32] = 1.0 / 32.0
    return ident, pmat


_PROGRAM_CACHE: dict[int, bass.Bass] = {}


def kernel(q: np.ndarray, k: np.ndarray, v: np.ndarray, BS) -> tuple:
    assert int(BS) == BS_EXPECTED, f"kernel hardcodes BS=32, got {BS}"
    q = np.ascontiguousarray(np.asarray(q, dtype=np.float32)).reshape(B * H, T, D)
    k = np.ascontiguousarray(np.asarray(k, dtype=np.float32)).reshape(B * H, T, D)
    v = np.ascontiguousarray(np.asarray(v, dtype=np.float32)).reshape(B * H, T, D)

    if 1 not in _PROGRAM_CACHE:
        _PROGRAM_CACHE[1] = build_program(reps=1)
    nc = _PROGRAM_CACHE[1]

    ident, _pmat = _make_const_inputs()
    in_maps = []
    for i in range(N_CORES):
        sl = slice(i * HEADS_PER_CORE, (i + 1) * HEADS_PER_CORE)
        in_maps.append({
            "q": q[sl], "k": k[sl], "v": v[sl],
            "ident": ident,
        })

    res = run_bass_kernel_spmd(nc, in_maps, core_ids=list(range(N_CORES)))

    out = np.empty((B * H, T, D), dtype=np.float32)
    score = np.empty((B * H, T, C), dtype=np.float32)
    for i in range(N_CORES):
        sl = slice(i * HEADS_PER_CORE, (i + 1) * HEADS_PER_CORE)
        # device layout [h, p, j, c] with t = 32p + j -> plain reshape
        o = res.results[i]["out"].astype(np.float32)
        s = res.results[i]["score"].astype(np.float32)
        out[sl] = o.reshape(HEADS_PER_CORE, T, D)
        score[sl] = s.reshape(HEADS_PER_CORE, T, C)
    return out.reshape(B, H, T, D), score.reshape(B, H, T, C)


# revision 12
# speedup vs baseline: 1.7213x; 1.7213x over previous
"""Blockwise (compressed-KV) attention on 8 Trainium2 NeuronCores.

Problem: q,k,v [B=4,H=16,T=4096,D=128] fp32, BS=32.
  k_cmp/v_cmp = blockwise mean-pool of k/v along T -> [B,H,C=128,D]
  score = softmax(q @ k_cmp^T / sqrt(D))   [B,H,T,C]
  out   = score @ v_cmp                    [B,H,T,D]
Returns (out, score), matching the reference.

Sharding: the 64 (b,h) pairs are split 8-per-core (pure data parallel, no
communication).  Each core runs an identical Bass/Tile program over its
8 heads.

v3 design (per head, per core):
  loads: k,v via SWDGE cast DMA f32->fp16 as [128, 32*128] (partition =
    block index, 16 KiB contiguous per partition in HBM); q f32 as
    [128, 32, 128] on the SP HWDGE ring (one 2 MiB DMA).
  pooling: OFF the PE - 5-round fp16 tree-add on DVE (2x packed mode),
    final round emits f32 sums.  The 1/32 is folded into the exp scale
    (k side) and into the v_cmp fp16 copy (v side).
  k_cmp^T via one PE transpose of ksum; fp16 operands for all matmuls.
  main loop over 8 groups of 512 q rows:
    4 PE transposes q -> qT PSUM (f32), evac to fp16 SBUF (DVE+ACT split),
    QK^T: 4 matmuls (qT tile stationary, k_cmpT moving) -> S [t,c]
    S^T:  1 matmul  (k_cmpT stationary, qT moving, N=512)
    exp via ACT with scale=1/(32 sqrt(D)): S -> fp16 expt, S^T -> fp16 expT
    rowsums: DVE segmented reduce on expt -> sums f32; reciprocal
    normalize: DVE fp16 (4x mode) -> score_stage
    PV: 4 matmuls (expT tile stationary, v_cmp fp16 moving) -> out [t,d]
    evac out with scale=recip (ACT) -> fp16 out_stage
  stores: one 1 MiB fp16 DMA each for score/out per head on the ACT ring,
    DRAM layout [p, s, c] (partition-major, contiguous 8 KiB per partition);
    the host transposes back to [t, c] and upcasts to f32.
"""
import math

import numpy as np

import concourse.bass as bass
import concourse.tile as tile
from concourse import mybir
from concourse.bass_utils import run_bass_kernel_spmd
from concourse.vector_clock import ScopedClock

B, H, T, D = 4, 16, 4096, 128
BS_EXPECTED = 32
C = T // BS_EXPECTED  # 128 compressed slots
N_CORES = 8
HEADS_PER_CORE = B * H // N_CORES  # 8
N_TT = T // 128  # 32 t-tiles of 128 rows per head
F32 = mybir.dt.float32
FP16 = mybir.dt.float16

# ---------------------------------------------------------------------------
# walrus in this toolchain rejects instructions carrying more than one sync
# wait.  Tile's scheduler freely emits several waits per instruction, and the
# kernel-tail drain accumulates one wait per outstanding semaphore.  Hoist all
# but one wait of every instruction onto dedicated same-engine NOPs placed
# immediately before it (same-engine program order keeps the semantics).
_MAX_WAITS = 1
_split_counter = [0]


def _split_multi_waits(ordered):
    for insts in ordered.values():
        expanded = []
        for inst in insts:
            si = inst.sync_info
            if si is not None and len(si.on_wait) > _MAX_WAITS:
                waits = list(si.on_wait)
                head, keep = waits[:-_MAX_WAITS], waits[-_MAX_WAITS:]
                for w in head:
                    _split_counter[0] += 1
                    expanded.append(mybir.InstNoOp(
                        name=f"waitsplit_{_split_counter[0]}",
                        ins=[], outs=[],
                        engine=inst.engine,
                        sync_info=mybir.SyncInfo(on_wait=[w], on_update=[]),
                        bass_nofuse=True,
                    ))
                inst.sync_info = mybir.SyncInfo(
                    on_wait=keep, on_update=list(si.on_update)
                )
            expanded.append(inst)
        insts[:] = expanded


_orig_lower_ordered = tile.TileContext._lower_ordered_insts


def _lower_ordered_split(self, ordered):
    _split_multi_waits(ordered)
    return _orig_lower_ordered(self, ordered)


tile.TileContext._lower_ordered_insts = _lower_ordered_split


def _drain_and_barrier_split(self, tick_clock, wait_clock):
    nc = self.nc
    drain_inst = nc.sync.drain()
    wait_clock.add_sem_waits(
        drain_inst.ins, ScopedClock({None: tick_clock.global_clock})
    )
    si = drain_inst.ins.sync_info
    waits = list(si.on_wait) if si is not None else []
    if len(waits) > _MAX_WAITS:
        drain_inst.ins.sync_info = mybir.SyncInfo(
            on_wait=waits[:_MAX_WAITS], on_update=list(si.on_update)
        )
        for i in range(_MAX_WAITS, len(waits), _MAX_WAITS):
            extra = nc.sync.drain()
            extra.ins.sync_info = mybir.SyncInfo(
                on_wait=waits[i : i + _MAX_WAITS], on_update=[]
            )
    nc.all_engine_barrier()
    assert self.sems is not None
    popped = nc._tile_sem_poison_stack.pop()
    assert popped is self._sem_poison
    nc.clear_and_free_semaphores(list(self.sems.allocated().values()))
    nc.all_engine_barrier()


tile.TileContext._drain_and_barrier = _drain_and_barrier_split
# ---------------------------------------------------------------------------


def _tree_pool(nc, pool, src16, final_dtype, tag):
    """5-round fp16 pairwise-add tree: [128, 32*D] -> [128, D] sum."""
    cur = src16
    n = 16 * D
    while n > D:
        nxt = pool.tile([128, n], FP16, tag=f"{tag}{n}")
        nc.vector.tensor_tensor(
            nxt, cur[:, 0:n], cur[:, n : 2 * n], mybir.AluOpType.add
        )
        cur = nxt
        n //= 2
    out = pool.tile([128, D], final_dtype, tag=f"{tag}f")
    nc.vector.tensor_tensor(
        out, cur[:, 0:D], cur[:, D : 2 * D], mybir.AluOpType.add
    )
    return out


def build_program(reps: int = 1, cast_loads: bool = True, q16: bool = True,
                  sums_on: str = "dve", norm_on: str = "dve",
                  dma_only: bool = False) -> bass.Bass:
    """Build the per-core Bass program.  `reps` repeats the whole computation
    (identical work, same outputs) for slope-based wall-clock timing."""
    nc = bass.Bass("TRN2", target_bir_lowering=False, debug=False,
                   num_devices=N_CORES)

    q_d = nc.dram_tensor("q", [HEADS_PER_CORE, T, D], F32, kind="ExternalInput").ap()
    k_d = nc.dram_tensor("k", [HEADS_PER_CORE, T, D], F32, kind="ExternalInput").ap()
    v_d = nc.dram_tensor("v", [HEADS_PER_CORE, T, D], F32, kind="ExternalInput").ap()
    ident_d = nc.dram_tensor("ident", [128, 128], F32, kind="ExternalInput").ap()
    # [p, s, c] partition-major layout; host transposes back to [t, c].
    out_d = nc.dram_tensor("out", [HEADS_PER_CORE, 128, N_TT, D], FP16,
                           kind="ExternalOutput").ap()
    score_d = nc.dram_tensor("score", [HEADS_PER_CORE, 128, N_TT, C], FP16,
                             kind="ExternalOutput").ap()

    inv_scale = 1.0 / (BS_EXPECTED * math.sqrt(D))  # 1/32 pool fold + 1/sqrt(d)

    with tile.TileContext(nc) as tc:
        with (
            tc.tile_pool(name="singles", bufs=1) as singles,
            tc.tile_pool(name="kv", bufs=2) as kv_pool,
            tc.tile_pool(name="tree", bufs=2) as tree_pool,
            tc.tile_pool(name="qp", bufs=2) as q_pool,
            tc.tile_pool(name="heads", bufs=2) as heads,
            tc.tile_pool(name="sb", bufs=4) as sb_pool,
            tc.tile_pool(name="stage", bufs=2) as stage,
            tc.tile_pool(name="small", bufs=4) as small_pool,
            tc.tile_pool(name="psA", bufs=2, space="PSUM") as psA,
            tc.tile_pool(name="psS", bufs=2, space="PSUM") as psS,
            tc.tile_pool(name="psST", bufs=2, space="PSUM") as psST,
            tc.tile_pool(name="psO", bufs=2, space="PSUM") as psO,
        ):
            ident = singles.tile([128, 128], F32)
            nc.sync.dma_start(out=ident, in_=ident_d)
            ident16 = singles.tile([128, 128], FP16)
            nc.vector.tensor_copy(ident16, ident)
            ones16 = singles.tile([128, 1], FP16)
            nc.vector.memset(ones16, 1.0)
            q_dt = FP16 if q16 else F32
            q_ident = ident16 if q16 else ident

            for _rep in range(reps):
                for h in range(HEADS_PER_CORE):
                    # ---- loads ------------------------------------------
                    if cast_loads:
                        k_sb = kv_pool.tile([128, BS_EXPECTED * D], FP16, tag="kv")
                        nc.gpsimd.dma_start(
                            out=k_sb,
                            in_=k_d[h].rearrange("(p j) d -> p (j d)", p=128),
                        )
                        v_sb = kv_pool.tile([128, BS_EXPECTED * D], FP16, tag="kv")
                        nc.gpsimd.dma_start(
                            out=v_sb,
                            in_=v_d[h].rearrange("(p j) d -> p (j d)", p=128),
                        )
                    else:
                        k_sb = kv_pool.tile([128, BS_EXPECTED * D], F32, tag="kv32")
                        nc.sync.dma_start(
                            out=k_sb,
                            in_=k_d[h].rearrange("(p j) d -> p (j d)", p=128),
                        )
                        v_sb = kv_pool.tile([128, BS_EXPECTED * D], F32, tag="kv32")
                        nc.sync.dma_start(
                            out=v_sb,
                            in_=v_d[h].rearrange("(p j) d -> p (j d)", p=128),
                        )
                    # contiguous load: partition p holds rows t = 32p..32p+31,
                    # so tile j is q rows {32p + j} and every downstream
                    # [p, j] layout is linear t-order (t = 32p + j).
                    q_sb = q_pool.tile([128, N_TT, D], q_dt, tag="q")
                    if q16:
                        nc.gpsimd.dma_start(
                            out=q_sb,
                            in_=q_d[h].rearrange("(p j) d -> p j d", p=128),
                        )
                    else:
                        nc.sync.dma_start(
                            out=q_sb,
                            in_=q_d[h].rearrange("(p j) d -> p j d", p=128),
                        )

                    # ---- pooling (DVE) ----------------------------------
                    if cast_loads:
                        ksum = _tree_pool(nc, tree_pool, k_sb, F32, "k")
                        vsum = _tree_pool(nc, tree_pool, v_sb, F32, "v")
                    else:
                        ksum = heads.tile([128, D], F32, tag="ks")
                        nc.vector.reduce_sum(
                            ksum,
                            k_sb.rearrange("p (j d) -> p d j",
                                           j=BS_EXPECTED, d=D),
                            axis=mybir.AxisListType.X,
                        )
                        vsum = heads.tile([128, D], F32, tag="vs")
                        nc.vector.reduce_sum(
                            vsum,
                            v_sb.rearrange("p (j d) -> p d j",
                                           j=BS_EXPECTED, d=D),
                            axis=mybir.AxisListType.X,
                        )
                    # v_cmp = vsum/32 in fp16 (folds the mean)
                    v_cmp = heads.tile([128, D], FP16, tag="vc")
                    nc.scalar.activation(
                        v_cmp, vsum, mybir.ActivationFunctionType.Copy,
                        scale=1.0 / BS_EXPECTED,
                    )
                    kt_ps = psA.tile([128, 512], F32, tag="a")
                    nc.tensor.transpose(kt_ps[:, 0:128], ksum, ident)
                    k_cmpT = heads.tile([128, C], FP16, tag="kc")
                    nc.scalar.copy(k_cmpT, kt_ps[:, 0:128])

                    score_stage = stage.tile([128, N_TT, C], FP16, tag="sc")
                    out_stage = stage.tile([128, N_TT, D], FP16, tag="ou")

                    if dma_only:
                        nc.vector.memset(score_stage[:, 0:1, 0:1], 0.5)
                        nc.vector.memset(out_stage[:, 0:1, 0:1], 0.25)
                    else:
                        for g in range(N_TT // 4):  # 8 groups of 512 rows
                            qT_ps = psA.tile([128, 512], q_dt, tag="a")
                            for j in range(4):
                                nc.tensor.transpose(
                                    qT_ps[:, 128 * j : 128 * (j + 1)],
                                    q_sb[:, 4 * g + j, :], q_ident,
                                )
                            qT = sb_pool.tile([128, 512], FP16, tag="qT")
                            nc.vector.tensor_copy(qT[:, 0:256], qT_ps[:, 0:256])
                            nc.scalar.copy(qT[:, 256:512], qT_ps[:, 256:512])

                            s_ps = psS.tile([128, 512], F32, tag="s")
                            for j in range(4):
                                nc.tensor.matmul(
                                    s_ps[:, 128 * j : 128 * (j + 1)],
                                    lhsT=qT[:, 128 * j : 128 * (j + 1)],
                                    rhs=k_cmpT,
                                    start=True, stop=True,
                                )
                            stp_ps = psST.tile([128, 512], F32, tag="st")
                            nc.tensor.matmul(
                                stp_ps, lhsT=k_cmpT, rhs=qT,
                                start=True, stop=True,
                            )
                            expt = sb_pool.tile([128, 512], FP16, tag="exp")
                            nc.scalar.activation(
                                expt, s_ps, mybir.ActivationFunctionType.Exp,
                                scale=inv_scale,
                            )
                            expT = sb_pool.tile([128, 512], FP16, tag="st2")
                            nc.scalar.activation(
                                expT, stp_ps, mybir.ActivationFunctionType.Exp,
                                scale=inv_scale,
                            )
                            recip = small_pool.tile([128, 4], F32, tag="recip")
                            if sums_on == "pe":
                                sums_ps = psST.tile([128, 4], F32, tag="st")
                                for j in range(4):
                                    nc.tensor.matmul(
                                        sums_ps[:, j : j + 1],
                                        lhsT=expT[:, 128 * j : 128 * (j + 1)],
                                        rhs=ones16,
                                        start=True, stop=True,
                                    )
                                nc.vector.reciprocal(recip, sums_ps)
                            else:
                                sums = small_pool.tile([128, 4], F32, tag="sums")
                                nc.vector.reduce_sum(
                                    sums,
                                    expt.rearrange("p (j c) -> p j c", j=4),
                                    axis=mybir.AxisListType.X,
                                )
                                nc.vector.reciprocal(recip, sums)

                            for j in range(4):
                                norm_eng = nc.vector
                                if norm_on == "gps" or (
                                    norm_on == "split" and j >= 2
                                ):
                                    norm_eng = nc.gpsimd
                                norm_eng.tensor_scalar_mul(
                                    score_stage[:, 4 * g + j, :],
                                    expt[:, 128 * j : 128 * (j + 1)],
                                    recip[:, j : j + 1],
                                )

                            o_ps = psO.tile([128, 512], F32, tag="o")
                            for j in range(4):
                                nc.tensor.matmul(
                                    o_ps[:, 128 * j : 128 * (j + 1)],
                                    lhsT=expT[:, 128 * j : 128 * (j + 1)],
                                    rhs=v_cmp,
                                    start=True, stop=True,
                                )
                            for j in range(4):
                                nc.scalar.activation(
                                    out_stage[:, 4 * g + j, :],
                                    o_ps[:, 128 * j : 128 * (j + 1)],
                                    mybir.ActivationFunctionType.Copy,
                                    scale=recip[:, j : j + 1],
                                )

                    # ---- stores (ACT ring), contiguous per partition ----
                    nc.scalar.dma_start(out=score_d[h], in_=score_stage)
                    nc.scalar.dma_start(out=out_d[h], in_=out_stage)
    return nc


def _make_const_inputs():
    ident = np.eye(128, dtype=np.float32)
    pmat = np.zeros((128, 4), dtype=np.float32)
    for t in range(128):
        pmat[t, t // 32] = 1.0 / 32.0
    return ident, pmat


_PROGRAM_CACHE: dict[int, bass.Bass] = {}


def kernel(q: np.ndarray, k: np.ndarray, v: np.ndarray, BS) -> tuple:
    assert int(BS) == BS_EXPECTED, f"kernel hardcodes BS=32, got {BS}"
    q = np.ascontiguousarray(np.asarray(q, dtype=np.float32)).reshape(B * H, T, D)
    k = np.ascontiguousarray(np.asarray(k, dtype=np.float32)).reshape(B * H, T, D)
    v = np.ascontiguousarray(np.asarray(v, dtype=np.float32)).reshape(B * H, T, D)

    if 1 not in _PROGRAM_CACHE:
        _PROGRAM_CACHE[1] = build_program(reps=1)
    nc = _PROGRAM_CACHE[1]

    ident, _pmat = _make_const_inputs()
    in_maps = []
    for i in range(N_CORES):
        sl = slice(i * HEADS_PER_CORE, (i + 1) * HEADS_PER_CORE)
        in_maps.append({
            "q": q[sl], "k": k[sl], "v": v[sl],
            "ident": ident,
        })

    res = run_bass_kernel_spmd(nc, in_maps, core_ids=list(range(N_CORES)))

    out = np.empty((B * H, T, D), dtype=np.float32)
    score = np.empty((B * H, T, C), dtype=np.float32)
    for i in range(N_CORES):
        sl = slice(i * HEADS_PER_CORE, (i + 1) * HEADS_PER_CORE)
        # device layout [h, p, j, c] with t = 32p + j -> plain reshape
        o = res.results[i]["out"].astype(np.float32)
        s = res.results[i]["score"].astype(np.float32)
        out[sl] = o.reshape(HEADS_PER_CORE, T, D)
        score[sl] = s.reshape(HEADS_PER_CORE, T, C)
    return out.reshape(B, H, T, D), score.reshape(B, H, T, C)
